# revision 32
# baseline (speedup 1.0000x reference)
"""AttentionAggregator2d Trainium2 kernel (8 NeuronCores, data-parallel over batch).

Reference semantics (per batch sample):
    zm [256, 4096];  q = Wq@zm+bq [32, 4096];  k likewise;  v = Wv@zm+bv [256, 4096]
    A = softmax_rows(q^T k)        # A[t, i] = exp(q_t.k_i) / sum_i' exp(q_t.k_i')
    out[c, i] = zc[c, i] + gamma * sum_t v[c, t] * A[t, i]

Device algorithm (per core = one sample):
  stage 1: q/k/v projections as fp16 matmuls (fp16 inputs enable the PE fast
    weight load; fp32 PSUM). q and k are written replicated x4 across SBUF
    partition quadrants so the K=32 score matmuls can be 4-way row-packed via
    tile_position. v^T [t, c] is built per t-tile (zm chunk as the stationary
    operand); bv enters through a K=1 ones-row matmul into the same PSUM
    accumulation group. Emission order: k for all chunks first (scores need all
    of k), then the first q/v chunks; the rest drains inside run-0 exp windows.
  stage 2 (8 runs x 4 t-tiles), one "produce unit" = one [128, 2048] PSUM score
    tile (4-way packed K=32 matmuls over 4 i-chunks of 512) followed by a single
    wide ScalarE exp -> P~ bf16 in a 12-slot SBUF ring, with accum_out yielding
    the softmax denominator D[t] for free. Wide exp instructions matter: the
    ACT instruction overhead is ~1.3us regardless of width.
    uT[t, c] = v^T[t, c] * (gamma / D[t])  (per-partition scalars, bf16).
    consume (interleaved 2 chains per produce unit so the in-order PE queue
    fills the exp wait): acc[c-tile, i-chunk] = sum_t uT[t,c]^T P~[t,i]
    accumulated in PSUM over the run's 4 t-tiles, spilled to SBUF fp32 by DVE
    (zc folded into the run-0 spill; output lands directly in [c, i] layout).
  No softmax max-subtraction: |S| <= ~45 for this distribution, exact in fp32.
  PSUM budget: score tile 4 banks + 4 accumulation chain banks = 8.
"""

import numpy as np

N = 4096          # tokens (64*64)
C = 256           # channels (CM == CC)
P = 32            # q/k projection channels
NG = 16           # stage-1 token chunks of 256
TT = 128          # t-tile size
NTT = N // TT     # 32 t-tiles
RUN = 4           # t-tiles per accumulation run
NRUN = NTT // RUN # 4 runs
IC = 512          # i-chunk (output token chunk)
NIC = N // IC     # 8
PBLK_SLOTS = 12   # P~ ring slots (bf16 [128, 4096] each) = 3 runs
B = 8             # batch == cores

_cache = {}


def _build_module(repeat=1, lesion=None):
    import concourse.bacc as bacc
    import concourse.tile as tile
    from concourse import mybir
    from contextlib import ExitStack

    f32 = mybir.dt.float32
    f32r = mybir.dt.float32r
    bf16 = mybir.dt.bfloat16
    f16 = mybir.dt.float16
    AF = mybir.ActivationFunctionType
    OP = mybir.AluOpType

    nc = bacc.Bacc(trn_type="TRN2", debug=False)

    zm_d = nc.dram_tensor("zm", [C, N], f16, kind="ExternalInput").ap()
    zc_d = nc.dram_tensor("zc", [C, N], f32, kind="ExternalInput").ap()
    wq_d = nc.dram_tensor("wq4", [C, 128], f16, kind="ExternalInput").ap()
    wk_d = nc.dram_tensor("wk4", [C, 128], f16, kind="ExternalInput").ap()
    wv_d = nc.dram_tensor("wvt", [C, C], f16, kind="ExternalInput").ap()
    bq_d = nc.dram_tensor("bq4", [128, 1], f32, kind="ExternalInput").ap()
    bk_d = nc.dram_tensor("bk4", [128, 1], f32, kind="ExternalInput").ap()
    bv_d = nc.dram_tensor("bvr", [1, C], f16, kind="ExternalInput").ap()
    gam_d = nc.dram_tensor("gam", [128, 1], f32, kind="ExternalInput").ap()
    one_d = nc.dram_tensor("ones", [1, 128], f16, kind="ExternalInput").ap()
    out_d = nc.dram_tensor("out", [C, N], f32, kind="ExternalOutput").ap()

    with tile.TileContext(nc) as tc, ExitStack() as ctx:
        consts = ctx.enter_context(tc.tile_pool(name="consts", bufs=1))
        zm_pool = ctx.enter_context(tc.tile_pool(name="zmp", bufs=NG))
        big = ctx.enter_context(tc.tile_pool(name="big", bufs=1))
        p_pool = ctx.enter_context(tc.tile_pool(name="pblk", bufs=PBLK_SLOTS))
        ut_pool = ctx.enter_context(tc.tile_pool(name="ut", bufs=PBLK_SLOTS))
        d_pool = ctx.enter_context(tc.tile_pool(name="dp", bufs=6))
        zc_pool = ctx.enter_context(tc.tile_pool(name="zcp", bufs=4))
        ps_s = ctx.enter_context(tc.tile_pool(name="ps_s", bufs=1, space="PSUM"))
        ps_acc = ctx.enter_context(tc.tile_pool(name="ps_acc", bufs=4, space="PSUM"))

        # ---- constants ----
        wq_sb = consts.tile([128, 256], f16, name="wq_sb")
        wk_sb = consts.tile([128, 256], f16, name="wk_sb")
        wv_sb = consts.tile([128, 512], f16, name="wv_sb")
        bq_sb = consts.tile([128, 1], f32, name="bq_sb")
        bk_sb = consts.tile([128, 1], f32, name="bk_sb")
        bv_sb = consts.tile([1, C], f16, name="bv_sb")
        one_sb = consts.tile([1, 128], f16, name="one_sb")
        gam_sb = consts.tile([128, 1], f32, name="gam_sb")
        for h in range(2):
            nc.sync.dma_start(out=wq_sb[:, h * 128:(h + 1) * 128],
                              in_=wq_d[h * 128:(h + 1) * 128, :])
            nc.sync.dma_start(out=wk_sb[:, h * 128:(h + 1) * 128],
                              in_=wk_d[h * 128:(h + 1) * 128, :])
            nc.sync.dma_start(out=wv_sb[:, h * 256:(h + 1) * 256],
                              in_=wv_d[h * 128:(h + 1) * 128, :])
        nc.sync.dma_start(out=bq_sb, in_=bq_d)
        nc.sync.dma_start(out=bk_sb, in_=bk_d)
        nc.sync.dma_start(out=bv_sb, in_=bv_d)
        nc.sync.dma_start(out=gam_sb, in_=gam_d)
        nc.sync.dma_start(out=one_sb, in_=one_d)

        # ---- persistent tiles ----
        q_rep = big.tile([128, N], f16, name="q_rep")
        k_rep = big.tile([128, N], f16, name="k_rep")
        vt = big.tile([128, NTT * C], bf16, name="vt")          # v^T per t-tile [t, c]
        acc_sb = big.tile([128, 2 * N], f32, name="acc_sb")     # out staging [c-tile, i]
        if lesion == "noconsume":
            nc.vector.memset(acc_sb, 0.0)

        # ---- stage 1 upfront: k for all chunks (scores need all of k),
        # q/v for the first chunks; the rest drains inside run-0/1 windows. ----
        for _rep in range(repeat):
          zm_tiles = []
          for g in range(NG):
            sl = slice(g * 256, (g + 1) * 256)
            zm_t = zm_pool.tile([128, 512], f16, name="zm_t", tag="zm")
            zm_tiles.append(zm_t)
            nc.sync.dma_start(out=zm_t[:, 0:256], in_=zm_d[0:128, sl])
            nc.gpsimd.dma_start(out=zm_t[:, 256:512], in_=zm_d[128:256, sl])
            # k (ScalarE evac with bias)
            psk = ps_acc.tile([128, 512], f32, name="psk", tag="acc")
            nc.tensor.matmul(psk[:, 0:256], wk_sb[:, 0:128],
                             zm_t[:, 0:256], start=True, stop=False)
            nc.tensor.matmul(psk[:, 0:256], wk_sb[:, 128:256],
                             zm_t[:, 256:512], start=False, stop=True)
            if g < 8:
                nc.scalar.activation(k_rep[:, sl], psk[:, 0:256], AF.Identity, bias=bk_sb)
            else:
                nc.vector.tensor_scalar_add(k_rep[:, sl], psk[:, 0:256], bk_sb)

          def emit_q(g):
            sl = slice(g * 256, (g + 1) * 256)
            zm_t = zm_tiles[g]
            psq = ps_acc.tile([128, 512], f32, name="psq", tag="acc")
            nc.tensor.matmul(psq[:, 0:256], wq_sb[:, 0:128],
                             zm_t[:, 0:256], start=True, stop=False)
            nc.tensor.matmul(psq[:, 0:256], wq_sb[:, 128:256],
                             zm_t[:, 256:512], start=False, stop=True)
            nc.vector.tensor_scalar_add(q_rep[:, sl], psq[:, 0:256], bq_sb)

          def emit_v(g):
            zm_t = zm_tiles[g]
            for s in range(2):
                tt = 2 * g + s
                psv = ps_acc.tile([128, 512], f32, name="psv", tag="acc")
                nc.tensor.matmul(psv[:, 0:256], one_sb,
                                 bv_sb, start=True, stop=False)
                nc.tensor.matmul(psv[:, 0:256], zm_t[:, s * 128:(s + 1) * 128],
                                 wv_sb[:, 0:256], start=False, stop=False)
                nc.tensor.matmul(psv[:, 0:256], zm_t[:, 256 + s * 128:256 + (s + 1) * 128],
                                 wv_sb[:, 256:512], start=False, stop=True)
                nc.vector.tensor_copy(vt[:, tt * C:(tt + 1) * C], psv[:, 0:256])

          for g in range(2):
            emit_q(g)
          for g in range(4):
            emit_v(g)
          # stage-1 leftovers drained inside run-0/1 produce windows, ordered by
          # the run that needs them (q before its run starts, v before it ends)
          leftovers = []
          for rr in range(1, NRUN):
            leftovers += [("q", 2 * rr), ("q", 2 * rr + 1)]
            if rr >= 2:
                leftovers += [("v", 2 * rr), ("v", 2 * rr + 1)]
          li = 0

          # ---- stage 2: per produce unit (t-tile half: 4-packed scores into a
          # [128,2048] PSUM tile + one wide exp), interleave two consume chains of
          # the previous run so PE fills the exp wait; spills to SBUF by DVE. ----
          chains = [(c, ic) for ic in range(NIC) for c in range(2)]

          def emit_chain(run, pts, uts, c, ic):
            a = ps_acc.tile([128, 512], f32, name="a_out", tag="acc")
            for tl in range(RUN):
                nc.tensor.matmul(a, uts[tl][:, c * 128:(c + 1) * 128],
                                 pts[tl][:, ic * IC:(ic + 1) * IC],
                                 start=(tl == 0), stop=(tl == RUN - 1))
            dst = acc_sb[:, c * N + ic * IC: c * N + (ic + 1) * IC]
            if run == 0:
                zcs = zc_pool.tile([128, 512], f32, name="zcs", tag="zc")
                nc.gpsimd.dma_start(out=zcs, in_=zc_d[c * 128:(c + 1) * 128,
                                                      ic * IC:(ic + 1) * IC])
                nc.vector.tensor_tensor(dst, a, zcs, op=OP.add)
            else:
                nc.vector.tensor_tensor(dst, a, dst, op=OP.add)
            if run == NRUN - 1:
                nc.gpsimd.dma_start(out=out_d[c * 128:(c + 1) * 128,
                                              ic * IC:(ic + 1) * IC], in_=dst)

          prev = None
          for run in range(NRUN):
            pts = []
            uts = []
            dcol = d_pool.tile([128, 2 * RUN], f32, name="dcol", tag="dcol")
            for u in range(2 * RUN):
                tl, half = u // 2, u % 2
                tt = run * RUN + tl
                if half == 0:
                    pt = p_pool.tile([128, N], bf16, name="pt", tag="pt")
                    pts.append(pt)
                s = ps_s.tile([128, 2048], f32, name="s_sc", tag="s")
                for r in range(4):
                    ic = half * 4 + r
                    nc.tensor.matmul(
                        s[:, r * 512:(r + 1) * 512],
                        q_rep[32 * r:32 * (r + 1), tt * TT:(tt + 1) * TT],
                        k_rep[32 * r:32 * (r + 1), ic * IC:(ic + 1) * IC],
                        start=True, stop=True, tile_position=(32 * r, 0),
                    )
                nc.scalar.activation(pts[tl][:, half * 2048:(half + 1) * 2048], s,
                                     AF.Exp, accum_out=dcol[:, tl * 2 + half:tl * 2 + half + 1])
                # after each half-run (2 t-tiles done), fold D and emit uT so the
                # next run's first chains never stall on the denominators
                if u % 4 == 3:
                    hb = u // 4  # half-run index
                    dview = dcol.rearrange("p (t h) -> p t h", h=2)
                    dsum = d_pool.tile([128, 2], f32, name="dsum", tag="dsum")
                    nc.vector.tensor_tensor(dsum, dview[:, 2 * hb:2 * hb + 2, 0],
                                            dview[:, 2 * hb:2 * hb + 2, 1], op=OP.add)
                    drec = d_pool.tile([128, 2], f32, name="drec", tag="drec")
                    nc.vector.reciprocal(drec, dsum)
                    for tl in (2 * hb, 2 * hb + 1):
                        tt = run * RUN + tl
                        ut = ut_pool.tile([128, C], bf16, name="ut", tag="ut")
                        uts.append(ut)
                        nc.vector.tensor_scalar(ut, vt[:, tt * C:(tt + 1) * C],
                                                drec[:, tl - 2 * hb:tl - 2 * hb + 1],
                                                gam_sb, op0=OP.mult, op1=OP.mult)
                # fill the exp wait with consume chains (prev run) or stage-1 leftovers
                if prev is not None and lesion != "noconsume":
                    emit_chain(run - 1, prev[0], prev[1], *chains[2 * u])
                    emit_chain(run - 1, prev[0], prev[1], *chains[2 * u + 1])
                else:
                    for _ in range(3):
                        if li < len(leftovers):
                            kind, g = leftovers[li]
                            (emit_q if kind == "q" else emit_v)(g)
                            li += 1
            while li < len(leftovers):
                kind, g = leftovers[li]
                (emit_q if kind == "q" else emit_v)(g)
                li += 1
            prev = (pts, uts)
          if lesion != "noconsume":
            for u in range(2 * RUN):
                emit_chain(NRUN - 1, prev[0], prev[1], *chains[2 * u])
                emit_chain(NRUN - 1, prev[0], prev[1], *chains[2 * u + 1])
          else:
                # keep the output written so the NEFF has a defined output
                for c in range(2):
                    nc.gpsimd.dma_start(out=out_d[c * 128:(c + 1) * 128, :],
                                        in_=acc_sb[:, c * N:(c + 1) * N])

    nc.compile()
    return nc


def _get_module(repeat=1, lesion=None):
    key = f"nc{repeat}_{lesion}"
    if key not in _cache:
        _cache[key] = _build_module(repeat, lesion)
    return _cache[key]


def _host_prep(Wq, bq, Wk, bk, Wv, bv, gamma):
    g = np.float32(np.asarray(gamma).reshape(-1)[0])
    wq4 = np.ascontiguousarray(np.tile(np.asarray(Wq).T.astype(np.float16), (1, 4)))
    wk4 = np.ascontiguousarray(np.tile(np.asarray(Wk).T.astype(np.float16), (1, 4)))
    wvt = np.ascontiguousarray(np.asarray(Wv).T.astype(np.float16))
    bq4 = np.ascontiguousarray(np.tile(np.asarray(bq).astype(np.float32), 4).reshape(128, 1))
    bk4 = np.ascontiguousarray(np.tile(np.asarray(bk).astype(np.float32), 4).reshape(128, 1))
    bvr = np.ascontiguousarray(np.asarray(bv).astype(np.float16).reshape(1, C))
    gam = np.full((128, 1), g, np.float32)
    ones = np.ones((1, 128), np.float16)
    return dict(wq4=wq4, wk4=wk4, wvt=wvt, bq4=bq4, bk4=bk4, bvr=bvr, gam=gam, ones=ones)


def kernel(zc, zm, Wq, bq, Wk, bk, Wv, bv, gamma):
    from concourse.bass_utils import run_bass_kernel_spmd

    zc = np.asarray(zc)
    zm = np.asarray(zm)
    b, c, w, h = zm.shape
    assert (b, c, w * h) == (B, C, N), (zm.shape,)
    nc = _get_module()
    shared = _host_prep(Wq, bq, Wk, bk, Wv, bv, gamma)
    zmf = np.ascontiguousarray(zm.reshape(B, C, N).astype(np.float16))
    zcf = np.ascontiguousarray(zc.reshape(B, C, N).astype(np.float32))
    in_maps = [dict(zm=zmf[i], zc=zcf[i], **shared) for i in range(B)]
    res = run_bass_kernel_spmd(nc, in_maps, core_ids=list(range(B)))
    out = np.stack([r["out"] for r in res.results], axis=0)
    return out.reshape(b, c, w, h).astype(np.asarray(zc).dtype)
